# revision 3
# baseline (speedup 1.0000x reference)
"""CrossAttention kernel for 8 TRN2 NeuronCores.

Problem: B=8, N=M=1024, d_model=1024, 16 heads x 64 dim_head.
    q = x @ Wq; k = ctx @ Wk; v = ctx @ Wv   (per batch)
    out = softmax(q k^T / sqrt(64)) v @ Wo + bo

Sharding: data-parallel over batch. Core b computes batch element b end to
end; weights are replicated. No collectives.

v2 changes over the v1 baseline:
  - x/context are transposed on the HOST (free) and DMA'd directly into the
    [D-on-partitions, seq] layout: the whole PE-transpose stage is gone.
  - Weights are pre-arranged on the host so every weight DMA is a single
    fully-contiguous per-partition block.
  - softmax-denominator reciprocal uses reciprocal_approx_fast (the exact
    DVE reciprocal is an iterative-divide op, ~8 cyc/elem).
  - oT eviction copies (PSUM->SBUF) moved to the Scalar engine, which has
    slack; the normalize multiply runs as two [128,512] DVE ops per head
    pair instead of four [64,512] ones.
  - output projection in two 512-wide column halves; Wo parks in cT's SBUF
    slot once cT is dead.

Per-core dataflow (fp32 storage; float32r matmul compute):
  stage 1a: qT = matmul(lhsT=Wq, rhs=xT) -> [INNER, N], all four quarters
           up front (ACT-free PE prologue), plus quarter-0 k/v projections.
  stages 1b+2 interleaved by head-pair quarter, as v1: per step (head
           pair, N-chunk, M-chunk): paired sT matmuls -> one exp ->
           pipelined oT accumulation with an appended ones column in v
           accumulating the softmax denominator.
  stage 3: out = matmul(lhsT=oT, rhs=Wo) + bo in [128n x 512d] chunks.
"""

import numpy as np

import concourse.bass as bass
import concourse.mybir as mybir
import concourse.tile as tile
from concourse import bacc
from concourse import bass_utils
from concourse.masks import make_identity

P = 128
B = 8
N = 1024          # query length
M = 1024          # kv length
D = 1024          # d_model
H = 16
DH = 64
INNER = H * DH    # 1024
SCALE = DH ** -0.5
N_CORES = 8

F32 = mybir.dt.float32
MMDT = mybir.dt.float32r  # PE compute dtype: 4x fp32 throughput


def _mm(nc, out, lhsT, rhs, start, stop):
    nc.tensor.matmul(out, lhsT, rhs, start=start, stop=stop)


def _build_body(tc, xt_d, ct_d, wq_d, wk_d, wv_d, wo_d, bo_d, out_d):
    nc = tc.nc
    EXP = mybir.ActivationFunctionType.Exp

    from contextlib import ExitStack
    ctx = ExitStack()

    const = ctx.enter_context(tc.tile_pool(name="const", bufs=1))
    ps_p = ctx.enter_context(tc.tile_pool(name="ps_p", bufs=2, space="PSUM"))
    ps_s = ctx.enter_context(tc.tile_pool(name="ps_s", bufs=2, space="PSUM"))
    ps_o = ctx.enter_context(tc.tile_pool(name="ps_o", bufs=2, space="PSUM"))
    # xT slot reused for oT after stage 1; cT slot reused for Wo in stage 3
    xop = ctx.enter_context(tc.tile_pool(name="xop", bufs=1))
    ctp = ctx.enter_context(tc.tile_pool(name="ctp", bufs=1))
    qtp = ctx.enter_context(tc.tile_pool(name="qtp", bufs=1))
    ktp = ctx.enter_context(tc.tile_pool(name="ktp", bufs=1))
    wp = ctx.enter_context(tc.tile_pool(name="wp", bufs=3))
    vp = ctx.enter_context(tc.tile_pool(name="vp", bufs=1))
    pTp = ctx.enter_context(tc.tile_pool(name="pTp", bufs=2))
    rcpp = ctx.enter_context(tc.tile_pool(name="rcpp", bufs=1))
    outp = ctx.enter_context(tc.tile_pool(name="outp", bufs=3))

    ident = const.tile([P, P], F32, tag="ident")
    make_identity(nc, ident)
    ones = const.tile([1, P], MMDT, tag="ones")
    nc.vector.tensor_scalar(ones, ident[0:1, :], 0.0, 1.0,
                            mybir.AluOpType.mult, mybir.AluOpType.add)
    bo_sb = const.tile([1, D], MMDT, tag="bo")
    nc.sync.dma_start(bo_sb, bo_d)

    # ---- transposed inputs straight from HBM (host pre-transposed) ---------
    xT = xop.tile([P, 8, N], MMDT, tag="xT")  # xT[pi, po, n] = x[n, po*128+pi]
    cT = ctp.tile([P, 8, M], MMDT, tag="cT", name="cT")
    for po4 in range(4):
        nc.sync.dma_start(xT[:, 2 * po4:2 * po4 + 2, :],
                          xt_d[:, 2 * po4:2 * po4 + 2, :])
    for po4 in range(4):
        nc.sync.dma_start(cT[:, 2 * po4:2 * po4 + 2, :],
                          ct_d[:, 2 * po4:2 * po4 + 2, :])

    # ---- stage 1a: q projection + quarter-0 k/v (ACT-free PE prologue) ----
    qT = qtp.tile([P, 8, N], MMDT, tag="qT")  # qT[pi, po, n] = q[n, po*128+pi]
    kT = ktp.tile([P, 8, M], MMDT, tag="kT")
    # v[pi, mo, h, 0:64] = v[mo*128+pi, h*64+:], col 64 = 1.0 (denominator)
    v = vp.tile([P, 8, H, DH + 1], MMDT, tag="v")
    # f32r memset fails ISA codegen; write the ones column as ident*0 + 1.0
    nc.vector.tensor_scalar(
        v[:, :, :, DH:DH + 1],
        ident.rearrange("p (a b c) -> p a b c", a=8, b=H, c=1),
        0.0, 1.0, mybir.AluOpType.mult, mybir.AluOpType.add)

    WQ = 256  # weight tile: quarter of INNER columns

    def q_proj_quarter(wh):
        wt = wp.tile([P, 8, WQ], MMDT, tag="w", name=f"wq_{wh}")
        nc.sync.dma_start(wt, wq_d[:, wh])
        for ic in range(2):
            icg = wh * 2 + ic
            for nf in range(2):
                ps = ps_p.tile([P, 512], F32, tag="p", name="ps_q")
                for po in range(8):
                    _mm(nc, ps, wt[:, po, ic * P:(ic + 1) * P],
                        xT[:, po, nf * 512:(nf + 1) * 512],
                        start=(po == 0), stop=(po == 7))
                nc.vector.tensor_copy(qT[:, icg, nf * 512:(nf + 1) * 512], ps)

    def kv_proj_jobs(wh):
        """Emitter thunks for quarter wh's k and v projections (uses cT)."""
        wkt = wp.tile([P, 8, WQ], MMDT, tag="w", name=f"wk_{wh}")
        nc.sync.dma_start(wkt, wk_d[:, wh])
        wvt = wp.tile([P, 8, WQ], MMDT, tag="w", name=f"wv_{wh}")
        nc.sync.dma_start(wvt, wv_d[:, wh])

        def k_group(ic, nfk):
            icg = wh * 2 + ic
            ps = ps_p.tile([P, 512], F32, tag="p", name="ps_k")
            for po in range(8):
                _mm(nc, ps, wkt[:, po, ic * P:(ic + 1) * P],
                    cT[:, po, nfk * 512:(nfk + 1) * 512],
                    start=(po == 0), stop=(po == 7))
            nc.vector.tensor_copy(kT[:, icg, nfk * 512:(nfk + 1) * 512], ps)

        def v_group(mc):
            ps = ps_p.tile([P, WQ], F32, tag="p", name="ps_v")
            for po in range(8):
                _mm(nc, ps, cT[:, po, mc * P:(mc + 1) * P], wvt[:, po, :],
                    start=(po == 0), stop=(po == 7))
            nc.vector.tensor_copy(
                v[:, mc, wh * 4:(wh + 1) * 4, 0:DH],
                ps.rearrange("p (h dh) -> p h dh", dh=DH),
            )

        jobs = []
        for ic in range(2):
            for nfk in range(2):
                jobs.append(lambda ic=ic, nfk=nfk: k_group(ic, nfk))
        for mc in range(8):
            jobs.append(lambda mc=mc: v_group(mc))
        return jobs

    for wh in range(4):
        q_proj_quarter(wh)
    pending_jobs = kv_proj_jobs(0)
    for job in pending_jobs:
        job()

    # ---- stage 2: attention, interleaved with next quarter's projections ----
    oT = xop.tile([P, 8, N], MMDT, tag="xT",
                  name="oT")  # oT[pi, po, n] = o[n, po*128+pi]

    def norm_a(ots, hp, nf):
        """Block end: reciprocal + ScalarE copies out of PSUM."""
        rcp = rcpp.tile([1, 1024], MMDT, tag="rcp", name=f"rcp_{hp}_{nf}")
        for hi in range(2):
            rs = hi * DH
            with nc.allow_low_precision(reason="f32r softmax denom recip"):
                nc.vector.reciprocal(rcp[0:1, hi * 512:(hi + 1) * 512],
                                     ots[hi][DH:DH + 1, :])
            nc.scalar.copy(
                oT[rs:rs + DH, hp, nf * 512:(nf + 1) * 512], ots[hi][0:DH, :])
        return rcp

    def norm_b(rcp, hp, nf):
        """Deferred one step: broadcast reciprocal on PE, multiply in place."""
        for hi in range(2):
            rs = hi * DH
            bc = ps_p.tile([P, 512], F32, tag="p", name="bc")
            _mm(nc, bc[0:DH, :], ones[0:1, 0:DH],
                rcp[0:1, hi * 512:(hi + 1) * 512], start=True, stop=True)
            oT_slice = oT[rs:rs + DH, hp, nf * 512:(nf + 1) * 512]
            nc.vector.tensor_mul(oT_slice, oT_slice, bc[0:DH, :])

    def emit_oT(ots, hp, nf, mc, pt):
        for hi in range(2):
            h = 2 * hp + hi
            _mm(nc, ots[hi][0:DH + 1, :], v[:, mc, h, :],
                pt[:, hi * 512:(hi + 1) * 512],
                start=(mc == 0), stop=(mc == 7))
        if mc == 7:
            return (norm_a(ots, hp, nf), hp, nf)
        return None

    wo_tiles = {}

    def wo_load(dh):
        if "wo" not in wo_tiles:
            wo_tiles["wo"] = ctp.tile([P, 2, 8, 512], MMDT, tag="cT",
                                      name="wo")
        nc.sync.dma_start(wo_tiles["wo"][:, dh], wo_d[:, dh])

    def out_chunk(dh, nc8):
        ps = ps_p.tile([P, 512], F32, tag="p", name="ps_out")
        wo = wo_tiles["wo"]
        for po in range(8):
            _mm(nc, ps, oT[:, po, nc8 * P:(nc8 + 1) * P], wo[:, dh, po, :],
                start=(po == 0), stop=False)
        _mm(nc, ps, ones[0:1, 0:P], bo_sb[0:1, dh * 512:(dh + 1) * 512],
            start=False, stop=True)
        ot = outp.tile([P, 512], F32, tag="out")
        nc.vector.tensor_copy(ot, ps)
        nc.sync.dma_start(out_d[nc8 * P:(nc8 + 1) * P,
                                dh * 512:(dh + 1) * 512], ot)

    carry = []
    for wh in range(4):
        if wh < 3:
            kv = kv_proj_jobs(wh + 1)
            if wh == 2:
                # k(icg=7) chunks are first needed at quarter-3 step 16;
                # keep them as quarter-3 filler
                carry = kv[2:4]
                kv = kv[0:2] + kv[4:]
            next_jobs = [((i + 1) / (len(kv) + 1), j) for i, j in enumerate(kv)]
        else:
            # quarter 3: prefetch Wo; chunks over n in [0,512) depend only on
            # the nf=0 blocks, whose last producer (pair 7, nf=0) ends at
            # step 24/32 -- emit them in the last quarter's tail.
            # carry jobs read cT; Wo reuses cT's slot, so its DMA only
            # starts after the last carried cT read. keep carry early.
            next_jobs = [(0.04, carry[0]), (0.16, carry[1])]
            next_jobs += [(0.22, lambda: wo_load(0)), (0.5, lambda: wo_load(1))]
            next_jobs.sort(key=lambda fj: fj[0])
            # 0.79 is load-bearing: jobs at a step run after that step's
            # pending norm_b flush, and pair 7's nf=0 normalize flushes at
            # the start of step 26 (= fraction 25.3/32). Earlier fractions
            # would read pre-normalization oT for head pair 7.
            next_jobs += [(0.79 + 0.02 * i,
                           (lambda nc8=nc8: out_chunk(0, nc8)))
                          for i, nc8 in enumerate(range(4))]
            # dh=1 chunks for the first N half need the same oT blocks plus
            # Wo half 1 (loaded at 0.5)
            next_jobs += [(0.88 + 0.02 * i,
                           (lambda nc8=nc8: out_chunk(1, nc8)))
                          for i, nc8 in enumerate(range(4))]
        steps = [(hp, nf, mc)
                 for hp in (2 * wh, 2 * wh + 1)
                 for nf in range(2)
                 for mc in range(8)]
        n_steps = len(steps)
        pending = None
        pending_norm = None
        ots_cur = None
        job_i = 0
        for si, (hp, nf, mc) in enumerate(steps):
            if mc == 0:
                ots_cur = [ps_o.tile([P, 512], F32, tag="o",
                                     name=f"ot_{wh}_{hp}_{nf}_{i}")
                           for i in range(2)]
            st = ps_s.tile([P, 1024], F32, tag="s", name="st")
            for hi in range(2):
                rs = hi * DH
                _mm(nc, st[:, hi * 512:(hi + 1) * 512],
                    kT[rs:rs + DH, hp, mc * P:(mc + 1) * P],
                    qT[rs:rs + DH, hp, nf * 512:(nf + 1) * 512],
                    start=True, stop=True)
            pt = pTp.tile([P, 1024], MMDT, tag="pT")
            nc.scalar.activation(pt, st, EXP, scale=SCALE)
            if pending_norm is not None:
                norm_b(*pending_norm)
                pending_norm = None
            while job_i < len(next_jobs) and \
                    next_jobs[job_i][0] * n_steps <= si + 1:
                next_jobs[job_i][1]()
                job_i += 1
            if pending is not None:
                pending_norm = emit_oT(*pending) or pending_norm
            pending = (ots_cur, hp, nf, mc, pt)
        pending_norm = emit_oT(*pending) or pending_norm
        if pending_norm is not None:
            norm_b(*pending_norm)
            pending_norm = None
        while job_i < len(next_jobs):
            next_jobs[job_i][1]()
            job_i += 1

    # ---- stage 3: remaining output chunks -----------------------------------
    for nc8 in range(4, 8):
        out_chunk(0, nc8)
        out_chunk(1, nc8)

    ctx.close()


_NC_CACHE = None


def build_nc():
    global _NC_CACHE
    if _NC_CACHE is not None:
        return _NC_CACHE
    nc = bacc.Bacc("TRN2", target_bir_lowering=False, debug=False,
                   num_devices=N_CORES)
    xt_d = nc.dram_tensor("xt", [P, 8, N], MMDT, kind="ExternalInput").ap()
    ct_d = nc.dram_tensor("ct", [P, 8, M], MMDT, kind="ExternalInput").ap()
    wq_d = nc.dram_tensor("Wq", [P, 4, 8, 256], MMDT, kind="ExternalInput").ap()
    wk_d = nc.dram_tensor("Wk", [P, 4, 8, 256], MMDT, kind="ExternalInput").ap()
    wv_d = nc.dram_tensor("Wv", [P, 4, 8, 256], MMDT, kind="ExternalInput").ap()
    wo_d = nc.dram_tensor("Wo", [P, 2, 8, 512], MMDT, kind="ExternalInput").ap()
    bo_d = nc.dram_tensor("bo", [1, D], MMDT, kind="ExternalInput").ap()
    out_d = nc.dram_tensor("out", [N, D], F32, kind="ExternalOutput").ap()

    with tile.TileContext(nc) as tc:
        _build_body(tc, xt_d, ct_d, wq_d, wk_d, wv_d, wo_d, bo_d, out_d)
    nc.compile()
    _NC_CACHE = nc
    return nc


def _t_seq(a):
    # [seq, D] -> [pi, po, seq] with D = po*128 + pi
    return np.ascontiguousarray(
        a.T.reshape(8, P, a.shape[0]).transpose(1, 0, 2))


def _t_w(w, ncol):
    # [D, OUT] -> [pi, OUT//ncol, po, ncol] with D = po*128 + pi
    nq = w.shape[1] // ncol
    return np.ascontiguousarray(
        w.reshape(8, P, nq, ncol).transpose(1, 2, 0, 3))


def make_in_maps(x, context, Wq, Wk, Wv, Wo, bo):
    f = lambda a: np.asarray(a, dtype=np.float32)
    x, context = f(x), f(context)
    Wq, Wk, Wv, Wo, bo = f(Wq), f(Wk), f(Wv), f(Wo), f(bo)
    wq_h = _t_w(Wq, 256)
    wk_h = _t_w(Wk, 256)
    wv_h = _t_w(Wv, 256)
    wo_h = _t_w(Wo, 512)
    bo_h = np.ascontiguousarray(bo.reshape(1, D))
    return [
        {"xt": _t_seq(x[b]), "ct": _t_seq(context[b]),
         "Wq": wq_h, "Wk": wk_h, "Wv": wv_h, "Wo": wo_h, "bo": bo_h}
        for b in range(B)
    ]


def run(in_maps, trace=False, **kw):
    nc = build_nc()
    return bass_utils.run_bass_kernel_spmd(
        nc, in_maps, core_ids=list(range(N_CORES)), trace=trace, **kw)


def kernel(x, context, Wq, Wk, Wv, Wo, bo):
    res = run(make_in_maps(x, context, Wq, Wk, Wv, Wo, bo))
    return np.stack([res.results[b]["out"] for b in range(B)], axis=0)
